# revision 20
# baseline (speedup 1.0000x reference)
"""Trainium2 Bass kernel for CalibConv (5x5 conv -> linear -> drift modulation).

Math: per kernel position p=(i,j) and class k:
    cmap[n,h,w,p,k] = sum_c x_pad[n,c,h+i,w+j] * Weff[k,c,p] + beff[k]
where Weff[k,c,p] = sum_o Wlin[k,o]*Wt[o,c,i,j], beff = Wlin@bias + blin.
Then per output pixel:
    asum = sum_p |cmap|, ysum = sum_p yofs*|cmap|, xsum = sum_p xofs*|cmap|,
    csum = sum_p cmap,  out = csum * exp(-0.5*sqrt(ysum^2+xsum^2)/asum)

Device strategy (per core; batch N=8 data-parallel over 8 cores), bf16 path.
v2: minimize DMA count and per-DMA trigger/settle latency (each HWDGE
DIRECT2D trigger costs ~0.65us of queue time and each completion semaphore
settles ~1us after the transfer).

  1. One host blob xpar [C, 144+3600]: par (weff|smat|actp) then padded x.
     3 input DMAs (sync ring carries the first + the region-A gather
     triggers; scalar ring gets the other two + region-B).
  2. PE warm-up: junk 512-col matmuls during the input wait (HAM throttles
     a cold PE ~2.7x; warm G matmuls finish sooner so the ACT evacuation
     — the G-phase bottleneck — starts earlier).
  3. G phase: 8 bf16 matmuls (512-col chunks, PSUM bank pairs) with
     stationary weff [C, 128]: rows 0..99 hold 4 blocks of 25 kernel
     positions [id k0 | id k1 | abs k0 | abs k1] (abs blocks repeat the
     same weights; the duplication is free: PE and ACT cost per column).
  4. Evacuation: ONE ACT Prelu per pair (per-row bias/alpha: id rows
     pass through, abs rows |G'+beff|).  DVE cannot abs on TRN2
     (abs_max is v4-only), and ACT cost is per-column, so the single
     mixed-row Prelu is optimal.
  5. Gather via an HBM round trip in 2 disjoint write+read column
     regions (split at output col 1792, 244-col halo), one b2 tensor
     each: write A (cols 0:2044) depends only on evac pairs 0-1 so it
     overlaps evac of pairs 2-3; region A runs on the sync ring, region
     B on the scalar ring.  100 contiguous 3.1-4.1KB descriptors per
     DMA (the HBM read side is descriptor-rate bound, big runs win).
     Rows pre-shifted by -(300b+60i+j) on write, read back at -(300b):
     bc[r, c] = ag[r, c + 60i + j].
  6. stats: 27 matmuls (stationary bc[:, 128s:+128], moving smat
     [128, 8]) -> ONE PSUM tile [128 pixel-in-chunk, 27, 8].  bc rows
     [96:128] are memset 1.0 (reads overwrite 96..99); smat[100, 6:8]
     = 25*beff so csum lands biased via the const-1 row 100.
  7. single epilogue over [C, 27, 2]: DVE reciprocal + ACT Square +
     bit-trick sqrt (scale folded into the Exp scale immediate) + one
     ACT Exp; only the "exp_and_others" table set is ever loaded, once,
     at startup.
  8. ONE output DMA: out [128 pixel-in-chunk, 27 chunk, 2] f32; host
     re-indexes flat 60-stride pixels to (h, w).
"""

import numpy as np
import ml_dtypes

import concourse.bacc as bacc
import concourse.mybir as mybir
from concourse import tile
from concourse.ap import AP
from concourse.bass_utils import run_bass_kernel_spmd

N_CORES = 8
C, H, W = 128, 56, 56
KS, PAD = 5, 2
HP, WP = H + 2 * PAD, W + 2 * PAD    # 60, 60
NPIX = HP * WP                       # 3600
GLEN = (H - 1) * WP + W              # 3356 flat output pixels (60-stride)
P25 = KS * KS
NCLS = 2

NDAT = 100                           # 4 blocks * 25 rows
NB = 3360                            # bc width
MARG = 1148                          # b2 left margin (max shift 300*3+244+4)
HALO = 244                           # max gather shift

X0 = 144                             # x offset inside the xpar blob
XPW = X0 + NPIX                      # 3744

GSPLIT = 1792                        # gather A/B split (output col)
WLA = GSPLIT + 252                   # 2044: write A src cols [0, 2044)
WLB = NPIX - GSPLIT                  # 1808: write B src cols [1792, 3600)
WKA = MARG + WLA + 4                 # 2296: b2a row pitch
WKB = MARG + WLB + 4                 # 2060
RLA = GSPLIT                         # read A cols [0, 1792)
RLB = GLEN - GSPLIT                  # 1564: read B cols [1792, 3356)

SCH = 128                            # stats chunk pixels
S_OFFS = [128 * s for s in range(26)] + [GLEN - 128]   # 27 chunks
NS = len(S_OFFS)

SQRT_MAGIC = 0x1FC00000
SQRT_SCALE = 0.97056278

F32 = mybir.dt.float32
BF16 = mybir.dt.bfloat16
I32 = mybir.dt.int32
AF = mybir.ActivationFunctionType
ALU = mybir.AluOpType

# input DMA slices of the xpar blob: (start, end, engine_name)
IN_SLICES = ((0, X0 + 1024, "sync"), (X0 + 1024, X0 + 2304, "scalar"),
             (X0 + 2304, XPW, "scalar"))
# G chunks in x flat cols
G_CHUNKS = [(512 * g, 512 * (g + 1)) for g in range(7)] + [(3584, 3600)]
PAIRS = ((0, 1024), (1024, 2048), (2048, 3072), (3072, 3600))
N_WARM = 5


def kernel_body(tc, xpar_d, b2a_d, b2b_d, out_d):
    nc = tc.nc
    with (
        tc.tile_pool(name="const", bufs=1) as cpool,
        tc.tile_pool(name="big", bufs=1) as bpool,
        tc.tile_pool(name="psg", bufs=2, space="PSUM") as psg_pool,
        tc.tile_pool(name="psw", bufs=1, space="PSUM") as psw_pool,
        tc.tile_pool(name="pss", bufs=1, space="PSUM") as pss_pool,
        tc.tile_pool(name="tmp", bufs=1) as tpool,
    ):
        # ---- startup ------------------------------------------------
        warm = cpool.tile([1, 1], F32)
        nc.vector.memset(warm[:], 1.0)
        wj = cpool.tile([C, 512], BF16)
        nc.vector.memset(wj[:].bitcast(I32), 0x3F803F80)

        xp = bpool.tile([C, XPW], BF16)
        bc = bpool.tile([C, NB], BF16)
        nc.vector.memset(bc[96:128, :].bitcast(I32), 0x3F803F80)

        # first ACT instr triggers the one table load (exp_and_others
        # covers parametric_relu, square and exp); runs on the ACT engine
        # concurrently with the DIRECT2D triggers on the scalar sequencer
        nc.scalar.activation(warm[:], warm[:], AF.Prelu, alpha=0.5)

        # input DMAs (par first so weff lands earliest)
        for (a, b, eng) in IN_SLICES:
            getattr(nc, eng).dma_start(xp[:, a:b], xpar_d[:, a:b])

        weff = xp[:, 0:128]
        smat = xp[:, 128:136]
        actp = xp[:, 136:140].bitcast(F32)    # [:,0]=prelu bias, [:,1]=alpha

        ps_stats = pss_pool.tile([C, NS, NCLS * 4], F32, name="pstats")
        ps_junk = psw_pool.tile([C, 512], F32, name="pjunk")

        # PE warm-up on junk while input streams in (HAM un-throttles
        # after sustained busy; a cold PE runs the G matmuls ~2.7x slow)
        for _ in range(N_WARM):
            nc.tensor.matmul(ps_junk[:], wj[:, 0:128], wj[:], start=True,
                             stop=True)

        # ---- G phase: matmuls + Prelu evacuation ---------------------
        ag = bpool.tile([C, NPIX], BF16)
        for pi, (p0, p1) in enumerate(PAIRS):
            ps = psg_pool.tile([C, 1024], F32, tag="psg")
            for c0, c1 in G_CHUNKS:
                if not (p0 <= c0 < p1):
                    continue
                nc.tensor.matmul(
                    ps[:, c0 - p0 : c1 - p0], weff, xp[:, X0 + c0 : X0 + c1],
                    start=True, stop=True,
                )
            nc.scalar.activation(
                ag[0:NDAT, p0:p1], ps[0:NDAT, 0 : p1 - p0],
                AF.Prelu, bias=actp[0:NDAT, 0:1], alpha=actp[0:NDAT, 1:2],
            )

        # ---- gather round trip: 2 disjoint write+read col regions ----
        # descriptor batches stripe across the 16 HW DMA queues by OUTER
        # AP dim index: reads keep the 25-entry dim outermost (a 4-entry
        # outer dim engages only 4 queues and runs ~4x slower).  All
        # triggers on the sync ring (scalar's ACT queue is busy, and a
        # scalar DIRECT2D measured ~2x slower to issue).
        # 2 writes (20-outer dims spread ~10 queues) + 2 reads with the
        # 25-outer p-major pattern; the SBUF read dst stays a monotonic
        # row walk because bc rows are laid out p-major (partition 4p+b
        # holds ag row 25b+p); smat rows are permuted to match host-side
        agt = ag[:].tensor
        bct = bc[:].tensor
        nc.sync.dma_start(
            AP(b2a_d.tensor, MARG, [(5 * WKA - 60, 20), (WKA - 1, 5), (1, WLA)]),
            AP(agt, 0, [(NPIX, NDAT), (1, WLA)]),
        )
        nc.sync.dma_start(
            AP(b2b_d.tensor, MARG, [(5 * WKB - 60, 20), (WKB - 1, 5), (1, WLB)]),
            AP(agt, GSPLIT, [(NPIX, NDAT), (1, WLB)]),
        )
        nc.sync.dma_start(
            AP(bct, 0, [(NB, NDAT), (1, RLA)]),
            AP(b2a_d.tensor, MARG, [(WKA, 25), (25 * WKA - 300, 4), (1, RLA)]),
        )
        nc.sync.dma_start(
            AP(bct, GSPLIT, [(NB, NDAT), (1, RLB)]),
            AP(b2b_d.tensor, MARG, [(WKB, 25), (25 * WKB - 300, 4), (1, RLB)]),
        )

        # ---- stats matmuls into one PSUM bank ------------------------
        for s in range(NS):
            nc.tensor.matmul(
                ps_stats[:, s, :],
                bc[:, S_OFFS[s] : S_OFFS[s] + SCH],
                smat,
                start=True,
                stop=True,
            )

        # ---- epilogue: out = csum * exp(-0.5*sqrt(ssum)/asum) --------
        # split into region groups so group A overlaps read/stats of B
        NSA = GSPLIT // SCH           # 14 chunks in region A
        rinv = tpool.tile([C, NS, NCLS], F32)
        yx2 = tpool.tile([C, NS, 4], F32)
        ssum = tpool.tile([C, NS, NCLS], F32)
        y0 = tpool.tile([C, NS, NCLS], F32)
        dr2 = tpool.tile([C, NS, NCLS], F32)
        ex = tpool.tile([C, NS, NCLS], F32)
        outv = tpool.tile([C, NS, NCLS], F32)
        for (ga, gb) in ((0, NSA), (NSA, NS)):
            nc.vector.reciprocal(rinv[:, ga:gb], ps_stats[:, ga:gb, 0:2])
            nc.scalar.activation(yx2[:, ga:gb], ps_stats[:, ga:gb, 2:6],
                                 AF.Square)
            nc.vector.tensor_add(ssum[:, ga:gb], yx2[:, ga:gb, 0:2],
                                 yx2[:, ga:gb, 2:4])
            # op0/op1 must share an ALU class, so the bit-trick sqrt
            # stays two instructions
            nc.vector.tensor_scalar(
                y0[:, ga:gb].bitcast(I32), ssum[:, ga:gb].bitcast(I32),
                1, None, ALU.arith_shift_right,
            )
            nc.vector.tensor_scalar(
                y0[:, ga:gb].bitcast(I32), y0[:, ga:gb].bitcast(I32),
                SQRT_MAGIC, None, ALU.add,
            )
            nc.vector.tensor_mul(dr2[:, ga:gb], y0[:, ga:gb],
                                 rinv[:, ga:gb])
            nc.scalar.activation(ex[:, ga:gb], dr2[:, ga:gb], AF.Exp,
                                 scale=-0.5 * SQRT_SCALE)
            nc.vector.tensor_mul(outv[:, ga:gb], ex[:, ga:gb],
                                 ps_stats[:, ga:gb, 6:8])
        nc.sync.dma_start(out_d[:, :, :], outv[:])


def build_program():
    nc = bacc.Bacc("TRN2", target_bir_lowering=False, debug=False)
    xpar_d = nc.dram_tensor("xpar", [C, XPW], BF16, kind="ExternalInput").ap()
    b2a_d = nc.dram_tensor("b2a", [NDAT, WKA], BF16, kind="Internal").ap()
    b2b_d = nc.dram_tensor("b2b", [NDAT, WKB], BF16, kind="Internal").ap()
    out_d = nc.dram_tensor("out", [C, NS, NCLS], F32, kind="ExternalOutput").ap()
    with tile.TileContext(nc) as tc:
        kernel_body(tc, xpar_d, b2a_d, b2b_d, out_d)
    nc.compile()
    return nc


def host_params(Wt, bias, Wlin, blin):
    """Fold conv weights + linear projection into the device param blob."""
    Wt = np.asarray(Wt, np.float32)
    bias = np.asarray(bias, np.float32)
    Wlin = np.asarray(Wlin, np.float32)
    blin = np.asarray(blin, np.float32)
    O = Wt.shape[0]
    Wp = Wt.reshape(O, C, P25)
    Weff = np.einsum("ko,ocp->kcp", Wlin, Wp).astype(np.float32)  # (2, C, 25)
    beff = (Wlin @ bias + blin).astype(np.float32)                # (2,)
    offs = np.arange(-PAD, PAD + 1, dtype=np.float32)

    par = np.zeros((C, X0), dtype=ml_dtypes.bfloat16)
    actp = np.zeros((C, 2), np.float32)
    actp[:, 1] = 1.0
    smat = np.zeros((C, 8), np.float32)
    smat[100, 6:8] = 25.0 * beff        # csum bias via const-1 row (part 100)
    for b in range(4):                   # blocks: id k0, id k1, abs k0, abs k1
        k = b % 2
        is_abs = b >= 2
        for p in range(P25):
            i, j = p // KS, p % KS
            r = 25 * b + p
            par[:, r] = Weff[k, :, p].astype(ml_dtypes.bfloat16)
            if is_abs:
                actp[r, 0] = beff[k]
                actp[r, 1] = -1.0
                smat[r, 0 + k] = 1.0        # asum
                smat[r, 2 + k] = offs[i]    # ysum
                smat[r, 4 + k] = offs[j]    # xsum
            else:
                smat[r, 6 + k] = 1.0        # csum
    # bc rows land p-major (partition 4p+b <- ag row 25b+p): permute smat
    # data rows to match; const-1 row 100 stays put
    perm = np.arange(C)
    for p in range(P25):
        for b in range(4):
            perm[4 * p + b] = 25 * b + p
    smat[:100] = smat[perm[:100]]
    par[:, 128:136] = smat.astype(ml_dtypes.bfloat16)
    par[:, 136:140] = actp.view(np.uint16).view(ml_dtypes.bfloat16)
    return par


_nc_cache = None
last_results = None  # BassKernelResults of the most recent run (for profiling)
_out_r = None
_out_s = None


def _out_index():
    global _out_r, _out_s
    if _out_r is None:
        hp, wp = np.meshgrid(np.arange(H), np.arange(W), indexing="ij")
        f = (hp * WP + wp).ravel()
        s = np.minimum(f // SCH, NS - 1)
        r = f - np.where(s < NS - 1, 128 * s, GLEN - 128)
        _out_r, _out_s = r, s
    return _out_r, _out_s


def kernel(x, Wt, bias, Wlin, blin):
    global _nc_cache, last_results
    x = np.asarray(x, np.float32)
    xpad = np.pad(x, ((0, 0), (0, 0), (PAD, PAD), (PAD, PAD)))
    xpad = xpad.reshape(N_CORES, C, NPIX).astype(ml_dtypes.bfloat16)
    par = host_params(Wt, bias, Wlin, blin)
    blob = np.empty((N_CORES, C, XPW), dtype=ml_dtypes.bfloat16)
    blob[:, :, :X0] = par[None]
    blob[:, :, X0:] = xpad
    if _nc_cache is None:
        _nc_cache = build_program()
    in_maps = [{"xpar": blob[n]} for n in range(N_CORES)]
    res = run_bass_kernel_spmd(_nc_cache, in_maps, list(range(N_CORES)))
    last_results = res
    r, s = _out_index()
    out = np.stack(
        [
            res.results[n]["out"][r, s, :].reshape(H, W, NCLS)
            for n in range(N_CORES)
        ]
    )
    return out.astype(np.float32)
